# revision 25
# baseline (speedup 1.0000x reference)
"""VQ codebook (vector-quantization) Trainium2 Bass kernel.

Data-parallel over batch: 8 NeuronCores x 4 batches each.
Per core: z shard [4, 256, 1024] (NCHW, hw flattened), codebook [1024, 256].

Numerics are engineered to reproduce the jax-on-neuron reference argmin
bitwise-faithfully (including fp32 tie-break behavior):
  d = fl(fl(sz + se) - 2*mm) ; argmin with first-index tie-break.
  - sz, se computed host-side with sequential fp32 accumulation (bitwise
    equal to the device reference's jnp.sum).
  - mm via PE fp32 matmul of z @ (2e)^T (exact 2x scaling; ~1.5e-8 noise).
  - T1 = fl(se + sz) on ACT (bitwise exact adds).
  - s = fl(2mm - T1) on DVE; argmax via reduce_max + max_index
    (max_index returns the first occurrence -> first-index tie-break).
"""
import sys

sys.path.insert(0, "/opt/trn_rl_repo")

import numpy as np

import concourse.bass as bass
import concourse.mybir as mybir
import concourse.tile as tile
from concourse.bass_utils import run_bass_kernel_spmd
from concourse.masks import make_identity

F32 = mybir.dt.float32
I32 = mybir.dt.int32
U32 = mybir.dt.uint32

B, D, H, W = 32, 256, 32, 32
HW = H * W            # 1024
K = 1024              # codebook size
NCORES = 8
BPC = B // NCORES     # batches per core = 4
NPC = BPC * HW        # rows per core = 4096
NT = NPC // 128       # n-tiles per core = 32 (8 per batch)


def _legalize_waits(nc, max_waits=1):
    """This walrus build rejects >1 sync-wait command per instruction.
    Move overflow waits onto injected same-engine NoOps."""
    nsplit = 0
    for f in nc.m.functions:
        for bb in f.blocks:
            out = []
            changed = False
            for ins in bb.instructions:
                si = ins.sync_info
                waits = list(si.on_wait) if si is not None and si.on_wait else []
                if len(waits) > max_waits:
                    move, keep = waits[:-max_waits], waits[-max_waits:]
                    for ci, w in enumerate(move):
                        nop = mybir.InstNoOp(name=f"{ins.name}-ws{ci}", ins=[], outs=[])
                        nop.engine = ins.engine
                        nop.sync_info = mybir.SyncInfo(on_wait=[w], on_update=[])
                        out.append(nop)
                        nsplit += 1
                    si.on_wait = keep
                    changed = True
                out.append(ins)
            if changed:
                bb.instructions = out
    return nsplit


def _build_nc():
    nc = bass.Bass()

    z_d = nc.declare_dram_parameter("zshard", [BPC, D, HW], F32, isOutput=False)
    zrows_d = nc.declare_dram_parameter("zrows", [NPC, D], F32, isOutput=False)
    e2T_d = nc.declare_dram_parameter("e2T", [D, K], F32, isOutput=False)
    etab_d = nc.declare_dram_parameter("etab", [K, D], F32, isOutput=False)
    sz_d = nc.declare_dram_parameter("szpt", [128, NT], F32, isOutput=False)
    se_d = nc.declare_dram_parameter("serow", [1, K], F32, isOutput=False)

    zq_d = nc.declare_dram_parameter("zq", [BPC, D, HW], F32, isOutput=True)
    idx_d = nc.declare_dram_parameter("idx", [NT, 128], I32, isOutput=True)
    loss_d = nc.declare_dram_parameter("loss", [NPC, D], F32, isOutput=True)

    with tile.TileContext(nc) as tc:
        with (
            tc.tile_pool(name="const", bufs=1) as const,
            tc.tile_pool(name="zin", bufs=2) as zin,
            tc.tile_pool(name="work", bufs=3) as work,
            tc.tile_pool(name="hold", bufs=8) as hold,
            tc.tile_pool(name="outb", bufs=8) as outb,
            tc.tile_pool(name="psmm", bufs=3, space="PSUM") as psmm,
            tc.tile_pool(name="psqt", bufs=2, space="PSUM") as psqt,
        ):
            # ---- one-time loads ----
            e2T_t = const.tile([128, 2, K], F32)       # [d-chunk partitions, chunk, k]
            nc.sync.dma_start(e2T_t[:, 0, :], e2T_d[0:128, :])
            nc.sync.dma_start(e2T_t[:, 1, :], e2T_d[128:256, :])
            sz_t = const.tile([128, NT], F32)
            nc.sync.dma_start(sz_t, sz_d[:])
            se_t = const.tile([128, K], F32)
            se_ap = se_d[:]
            nc.gpsimd.dma_start(
                out=se_t,
                in_=bass.AP(tensor=se_ap.tensor, offset=se_ap.offset, ap=[[0, 128], [1, K]]),
            )
            ident = const.tile([128, 128], F32)
            make_identity(nc, ident)
            idxi_all = const.tile([128, NT], I32)      # argmin indices (int32)

            # PE warm-up: dummy matmuls on the identity while input DMAs land.
            # Keeps the PE busy from t=0 so HAM reaches K=8/8 before real work.
            warm_ps = psqt.tile([128, 512], F32, tag="qt")
            for _ in range(24):
                nc.tensor.matmul(warm_ps[:, :128], lhsT=ident[:], rhs=ident[:],
                                 start=True, stop=True)

            # back-end (gather-dependent) stage, deferred DELAY tiles so the
            # PE/GPSIMD never stall on the gather latency
            DELAY = 5
            pending = []

            def flush_backend():
                t, b, j, zq_sb, zrows = pending.pop(0)
                js = slice(j * 128, (j + 1) * 128)
                # loss = (zq - z)^2 on GPSIMD
                lsub = outb.tile([128, D], F32, tag="lsub")
                nc.gpsimd.tensor_tensor(out=lsub, in0=zq_sb, in1=zrows,
                                        op=mybir.AluOpType.subtract)
                loss_sb = outb.tile([128, D], F32, tag="loss")
                nc.scalar.activation(out=loss_sb, in_=lsub,
                                     func=mybir.ActivationFunctionType.Square)
                nc.sync.dma_start(loss_d[t * 128:(t + 1) * 128, :], loss_sb)
                # z_q NCHW via PE transpose; both chunks -> one DMA
                zqT = outb.tile([128, 2, 128], F32, tag="zqT")
                for c in range(2):
                    qt_ps = psqt.tile([128, 128], F32, tag="qt")
                    nc.tensor.transpose(qt_ps[:], zq_sb[:, c * 128:(c + 1) * 128], ident[:])
                    nc.scalar.activation(out=zqT[:, c, :], in_=qt_ps,
                                         func=mybir.ActivationFunctionType.Copy)
                zq_ap = zq_d[:]
                dst = bass.AP(tensor=zq_ap.tensor,
                              offset=zq_ap.offset + b * D * HW + j * 128,
                              ap=[[HW, 128], [128 * HW, 2], [1, 128]])
                nc.sync.dma_start(dst, zqT[:])

            for b in range(BPC):
                # z natural chunks for this batch: [128 (d), HW] x2
                zc = zin.tile([128, 2, HW], F32)
                nc.sync.dma_start(zc[:, 0, :], z_d[b, 0:128, :])
                nc.sync.dma_start(zc[:, 1, :], z_d[b, 128:256, :])

                for j in range(HW // 128):
                    t = b * (HW // 128) + j
                    js = slice(j * 128, (j + 1) * 128)

                    # ---- scores: psum = z @ (2e)^T  [128 n, 1024 k] ----
                    mm_ps = psmm.tile([128, K], F32)
                    for c in range(2):
                        for h in range(2):
                            nc.tensor.matmul(
                                mm_ps[:, h * 512:(h + 1) * 512],
                                lhsT=zc[:, c, js],
                                rhs=e2T_t[:, c, h * 512:(h + 1) * 512],
                                start=(c == 0),
                                stop=(c == 1),
                            )

                    # flush one deferred backend tile before emitting this
                    # tile's ACT work, so zqT copies sit ahead of T1 in the
                    # ACT queue (PE's qt-psum slot depends on those copies)
                    if len(pending) > DELAY:
                        flush_backend()

                    # ---- z rows (n, d): host-transposed input, 2 tiles/DMA ----
                    if t % 2 == 0:
                        zrows2 = hold.tile([128, 2, D], F32, tag="zrows")
                        nc.sync.dma_start(zrows2, zrows_d[t * 128:(t + 2) * 128, :].rearrange("(i p) d -> p i d", p=128))
                    zrows = zrows2[:, t % 2, :]

                    # ---- T1 = fl(se + sz[n]) on ACT ----
                    t1 = work.tile([128, K], F32)
                    nc.scalar.activation(out=t1, in_=se_t,
                                         func=mybir.ActivationFunctionType.Identity,
                                         bias=sz_t[:, t:t + 1], scale=1.0)

                    # ---- s = fl(2mm - T1); argmax (= argmin d, first-index ties) ----
                    # split into k-halves so DVE starts while PE finishes half 1
                    s_sb = work.tile([128, K], F32)
                    smax2 = work.tile([128, 2], F32)
                    for h in range(2):
                        hs = slice(h * 512, (h + 1) * 512)
                        nc.vector.tensor_tensor(out=s_sb[:, hs], in0=mm_ps[:, hs],
                                                in1=t1[:, hs], op=mybir.AluOpType.subtract)
                        nc.vector.tensor_reduce(out=smax2[:, h:h + 1], in_=s_sb[:, hs],
                                                axis=mybir.AxisListType.X,
                                                op=mybir.AluOpType.max)
                    smax = work.tile([128, 1], F32)
                    nc.vector.tensor_tensor(out=smax, in0=smax2[:, 0:1], in1=smax2[:, 1:2],
                                            op=mybir.AluOpType.max)
                    idx8 = work.tile([128, 8], U32)
                    nc.vector.max_index(idx8, smax[:, :1].to_broadcast([128, 8]), s_sb)
                    nc.gpsimd.tensor_copy(idxi_all[:, t:t + 1], idx8[:, 0:1])

                    # ---- gather z_q rows from the codebook (async wrt backend) ----
                    zq_sb = outb.tile([128, D], F32, tag="zq")
                    nc.gpsimd.indirect_dma_start(
                        out=zq_sb[:], out_offset=None, in_=etab_d[:],
                        in_offset=bass.IndirectOffsetOnAxis(ap=idx8[:, 0:1], axis=0),
                    )
                    pending.append((t, b, j, zq_sb, zrows))

            while pending:
                flush_backend()

            # ---- indices out: cast to f32, transpose [128, NT] -> [NT, 128] ----
            idxf_all = const.tile([128, NT], F32)
            nc.vector.tensor_copy(idxf_all, idxi_all)
            idxT_ps = psqt.tile([128, 128], F32, tag="qt")
            nc.tensor.transpose(idxT_ps[:NT, :], idxf_all[:], ident[:])
            idxT = const.tile([NT, 128], I32)
            nc.vector.tensor_copy(idxT, idxT_ps[:NT, :])
            nc.sync.dma_start(idx_d[:], idxT)

    _legalize_waits(nc)
    return nc


_NC = None


def _get_nc():
    global _NC
    if _NC is None:
        _NC = _build_nc()
    return _NC


def _seqsum_sq_f32(x):
    """Sequential fp32 sum of squares along the last axis (matches the
    on-device jnp.sum reduction order bitwise)."""
    x = np.ascontiguousarray(x, dtype=np.float32)
    acc = np.zeros(x.shape[:-1], np.float32)
    for i in range(x.shape[-1]):
        acc = (acc + x[..., i] * x[..., i]).astype(np.float32)
    return acc


def kernel(z, embedding_weight):
    z = np.ascontiguousarray(np.asarray(z, dtype=np.float32))
    e = np.ascontiguousarray(np.asarray(embedding_weight, dtype=np.float32))
    assert z.shape == (B, D, H, W) and e.shape == (K, D)

    nc = _get_nc()

    # host-side exact prep
    e2T = np.ascontiguousarray((np.float32(2.0) * e).T)            # [256, 1024]
    se = _seqsum_sq_f32(e)[None, :].astype(np.float32)             # [1, 1024]
    z_flat = z.reshape(B, D, HW)

    in_maps = []
    for i in range(NCORES):
        zs = np.ascontiguousarray(z_flat[i * BPC:(i + 1) * BPC])   # [4, 256, 1024]
        # sz for rows n=(b,hw) in NHWC order: z^T rows; layout [128, NT] (p, t)
        zrows = np.ascontiguousarray(zs.transpose(0, 2, 1).reshape(NPC, D))
        sz = _seqsum_sq_f32(zrows)                                 # [4096]
        szpt = np.ascontiguousarray(sz.reshape(NT, 128).T)         # [128, NT]
        in_maps.append({
            "zshard": zs,
            "zrows": zrows,
            "e2T": e2T,
            "etab": e,
            "szpt": szpt,
            "serow": se,
        })

    res = run_bass_kernel_spmd(nc, in_maps, list(range(NCORES))).results

    zq = np.concatenate([r["zq"].reshape(BPC, D, H, W) for r in res], axis=0)
    idx = np.concatenate([r["idx"].reshape(NPC) for r in res], axis=0).astype(np.int32)
    loss = np.concatenate([r["loss"].reshape(BPC, H, W, D) for r in res], axis=0)
    return zq, idx, loss


# revision 27
# speedup vs baseline: 1.1311x; 1.1311x over previous
"""VQ codebook (vector-quantization) Trainium2 Bass kernel.

Data-parallel over batch: 8 NeuronCores x 4 batches each.
Per core: z shard [4, 256, 1024] (NCHW, hw flattened), codebook [1024, 256].

Numerics are engineered to reproduce the jax-on-neuron reference argmin
bitwise-faithfully (including fp32 tie-break behavior):
  d = fl(fl(sz + se) - 2*mm) ; argmin with first-index tie-break.
  - sz, se computed host-side with sequential fp32 accumulation (bitwise
    equal to the device reference's jnp.sum).
  - mm via PE fp32 matmul of z @ (2e)^T (exact 2x scaling; ~1.5e-8 noise).
  - T1 = fl(se + sz) on ACT (bitwise exact adds).
  - s = fl(2mm - T1) on DVE; argmax via reduce_max + max_index
    (max_index returns the first occurrence -> first-index tie-break).
"""
import sys

sys.path.insert(0, "/opt/trn_rl_repo")

import numpy as np

import concourse.bass as bass
import concourse.mybir as mybir
import concourse.tile as tile
from concourse.bass_utils import run_bass_kernel_spmd
from concourse.masks import make_identity

F32 = mybir.dt.float32
I32 = mybir.dt.int32
U32 = mybir.dt.uint32

B, D, H, W = 32, 256, 32, 32
HW = H * W            # 1024
K = 1024              # codebook size
NCORES = 8
BPC = B // NCORES     # batches per core = 4
NPC = BPC * HW        # rows per core = 4096
NT = NPC // 128       # n-tiles per core = 32 (8 per batch)


def _legalize_waits(nc, max_waits=1):
    """This walrus build rejects >1 sync-wait command per instruction.
    Move overflow waits onto injected same-engine NoOps."""
    nsplit = 0
    for f in nc.m.functions:
        for bb in f.blocks:
            out = []
            changed = False
            for ins in bb.instructions:
                si = ins.sync_info
                waits = list(si.on_wait) if si is not None and si.on_wait else []
                if len(waits) > max_waits:
                    move, keep = waits[:-max_waits], waits[-max_waits:]
                    for ci, w in enumerate(move):
                        nop = mybir.InstNoOp(name=f"{ins.name}-ws{ci}", ins=[], outs=[])
                        nop.engine = ins.engine
                        nop.sync_info = mybir.SyncInfo(on_wait=[w], on_update=[])
                        out.append(nop)
                        nsplit += 1
                    si.on_wait = keep
                    changed = True
                out.append(ins)
            if changed:
                bb.instructions = out
    return nsplit


def _build_nc():
    nc = bass.Bass()

    z_d = nc.declare_dram_parameter("zshard", [BPC, D, HW], F32, isOutput=False)
    zrows_d = nc.declare_dram_parameter("zrows", [NPC, D], F32, isOutput=False)
    e2T_d = nc.declare_dram_parameter("e2T", [D, K], F32, isOutput=False)
    etab_d = nc.declare_dram_parameter("etab", [K, D], F32, isOutput=False)
    sz_d = nc.declare_dram_parameter("szpt", [128, NT], F32, isOutput=False)
    se_d = nc.declare_dram_parameter("serow", [1, K], F32, isOutput=False)

    zq_d = nc.declare_dram_parameter("zq", [BPC, D, HW], F32, isOutput=True)
    idx_d = nc.declare_dram_parameter("idx", [NT, 128], I32, isOutput=True)
    loss_d = nc.declare_dram_parameter("loss", [NPC, D], F32, isOutput=True)

    with tile.TileContext(nc) as tc:
        with (
            tc.tile_pool(name="const", bufs=1) as const,
            tc.tile_pool(name="zin", bufs=2) as zin,
            tc.tile_pool(name="work", bufs=3) as work,
            tc.tile_pool(name="hold", bufs=8) as hold,
            tc.tile_pool(name="outb", bufs=8) as outb,
            tc.tile_pool(name="psmm", bufs=3, space="PSUM") as psmm,
            tc.tile_pool(name="psqt", bufs=2, space="PSUM") as psqt,
        ):
            # ---- one-time loads ----
            e2T_t = const.tile([128, 2, K], F32)       # [d-chunk partitions, chunk, k]
            nc.sync.dma_start(e2T_t[:, 0, :], e2T_d[0:128, :])
            nc.sync.dma_start(e2T_t[:, 1, :], e2T_d[128:256, :])
            sz_t = const.tile([128, NT], F32)
            nc.sync.dma_start(sz_t, sz_d[:])
            se_t = const.tile([128, K], F32)
            se_ap = se_d[:]
            nc.gpsimd.dma_start(
                out=se_t,
                in_=bass.AP(tensor=se_ap.tensor, offset=se_ap.offset, ap=[[0, 128], [1, K]]),
            )
            ident = const.tile([128, 128], F32)
            make_identity(nc, ident)
            idxi_all = const.tile([128, NT], I32)      # argmin indices (int32)

            # PE warm-up: dummy matmuls on the identity while input DMAs land.
            # Keeps the PE busy from t=0 so HAM reaches K=8/8 before real work.
            warm_ps = psqt.tile([128, 512], F32, tag="qt")
            for _ in range(24):
                nc.tensor.matmul(warm_ps[:, :128], lhsT=ident[:], rhs=ident[:],
                                 start=True, stop=True)

            # back-end (gather-dependent) stage, deferred DELAY tiles so the
            # PE/GPSIMD never stall on the gather latency
            DELAY = 5
            pending = []

            def flush_backend():
                t, b, j, zq_sb, zrows = pending.pop(0)
                js = slice(j * 128, (j + 1) * 128)
                # loss = (zq - z)^2 on GPSIMD
                lsub = outb.tile([128, D], F32, tag="lsub")
                nc.gpsimd.tensor_tensor(out=lsub, in0=zq_sb, in1=zrows,
                                        op=mybir.AluOpType.subtract)
                loss_sb = outb.tile([128, D], F32, tag="loss")
                nc.scalar.activation(out=loss_sb, in_=lsub,
                                     func=mybir.ActivationFunctionType.Square)
                nc.sync.dma_start(loss_d[t * 128:(t + 1) * 128, :], loss_sb)
                # z_q NCHW via PE transpose; both chunks -> one DMA
                zqT = outb.tile([128, 2, 128], F32, tag="zqT")
                for c in range(2):
                    qt_ps = psqt.tile([128, 128], F32, tag="qt")
                    nc.tensor.transpose(qt_ps[:], zq_sb[:, c * 128:(c + 1) * 128], ident[:])
                    nc.scalar.activation(out=zqT[:, c, :], in_=qt_ps,
                                         func=mybir.ActivationFunctionType.Copy)
                zq_ap = zq_d[:]
                dst = bass.AP(tensor=zq_ap.tensor,
                              offset=zq_ap.offset + b * D * HW + j * 128,
                              ap=[[HW, 128], [128 * HW, 2], [1, 128]])
                nc.sync.dma_start(dst, zqT[:])

            for b in range(BPC):
                # z natural chunks for this batch: [128 (d), HW] x2
                zc = zin.tile([128, 2, HW], F32)
                nc.sync.dma_start(zc[:, 0, :], z_d[b, 0:128, :])
                nc.sync.dma_start(zc[:, 1, :], z_d[b, 128:256, :])

                for j in range(HW // 128):
                    t = b * (HW // 128) + j
                    js = slice(j * 128, (j + 1) * 128)

                    # ---- scores: psum = z @ (2e)^T  [128 n, 1024 k] ----
                    mm_ps = psmm.tile([128, K], F32)
                    for c in range(2):
                        for h in range(2):
                            nc.tensor.matmul(
                                mm_ps[:, h * 512:(h + 1) * 512],
                                lhsT=zc[:, c, js],
                                rhs=e2T_t[:, c, h * 512:(h + 1) * 512],
                                start=(c == 0),
                                stop=(c == 1),
                            )

                    # ---- z rows (n, d): host-transposed input, 2 tiles/DMA ----
                    if t % 2 == 0:
                        zrows2 = hold.tile([128, 2, D], F32, tag="zrows")
                        nc.sync.dma_start(zrows2, zrows_d[t * 128:(t + 2) * 128, :].rearrange("(i p) d -> p i d", p=128))
                    zrows = zrows2[:, t % 2, :]

                    # ---- T1 = fl(se + sz[n]) on ACT ----
                    t1 = work.tile([128, K], F32)
                    nc.scalar.activation(out=t1, in_=se_t,
                                         func=mybir.ActivationFunctionType.Identity,
                                         bias=sz_t[:, t:t + 1], scale=1.0)

                    # ---- s = fl(2mm - T1); argmax (= argmin d, first-index ties) ----
                    # split into k-halves so DVE starts while PE finishes half 1
                    s_sb = work.tile([128, K], F32)
                    smax2 = work.tile([128, 2], F32)
                    for h in range(2):
                        hs = slice(h * 512, (h + 1) * 512)
                        nc.vector.tensor_tensor(out=s_sb[:, hs], in0=mm_ps[:, hs],
                                                in1=t1[:, hs], op=mybir.AluOpType.subtract)
                        nc.vector.tensor_reduce(out=smax2[:, h:h + 1], in_=s_sb[:, hs],
                                                axis=mybir.AxisListType.X,
                                                op=mybir.AluOpType.max)
                    smax = work.tile([128, 1], F32)
                    nc.vector.tensor_tensor(out=smax, in0=smax2[:, 0:1], in1=smax2[:, 1:2],
                                            op=mybir.AluOpType.max)
                    idx8 = work.tile([128, 8], U32)
                    nc.vector.max_index(idx8, smax[:, :1].to_broadcast([128, 8]), s_sb)
                    nc.gpsimd.tensor_copy(idxi_all[:, t:t + 1], idx8[:, 0:1])

                    # ---- gather z_q rows from the codebook (async wrt backend) ----
                    zq_sb = outb.tile([128, D], F32, tag="zq")
                    nc.gpsimd.indirect_dma_start(
                        out=zq_sb[:], out_offset=None, in_=etab_d[:],
                        in_offset=bass.IndirectOffsetOnAxis(ap=idx8[:, 0:1], axis=0),
                    )
                    pending.append((t, b, j, zq_sb, zrows))
                    if len(pending) > DELAY:
                        flush_backend()

            while pending:
                flush_backend()

            # ---- indices out: cast to f32, transpose [128, NT] -> [NT, 128] ----
            idxf_all = const.tile([128, NT], F32)
            nc.vector.tensor_copy(idxf_all, idxi_all)
            idxT_ps = psqt.tile([128, 128], F32, tag="qt")
            nc.tensor.transpose(idxT_ps[:NT, :], idxf_all[:], ident[:])
            idxT = const.tile([NT, 128], I32)
            nc.vector.tensor_copy(idxT, idxT_ps[:NT, :])
            nc.sync.dma_start(idx_d[:], idxT)

    _legalize_waits(nc)
    return nc


_NC = None


def _get_nc():
    global _NC
    if _NC is None:
        _NC = _build_nc()
    return _NC


def _seqsum_sq_f32(x):
    """Sequential fp32 sum of squares along the last axis (matches the
    on-device jnp.sum reduction order bitwise)."""
    x = np.ascontiguousarray(x, dtype=np.float32)
    acc = np.zeros(x.shape[:-1], np.float32)
    for i in range(x.shape[-1]):
        acc = (acc + x[..., i] * x[..., i]).astype(np.float32)
    return acc


def kernel(z, embedding_weight):
    z = np.ascontiguousarray(np.asarray(z, dtype=np.float32))
    e = np.ascontiguousarray(np.asarray(embedding_weight, dtype=np.float32))
    assert z.shape == (B, D, H, W) and e.shape == (K, D)

    nc = _get_nc()

    # host-side exact prep
    e2T = np.ascontiguousarray((np.float32(2.0) * e).T)            # [256, 1024]
    se = _seqsum_sq_f32(e)[None, :].astype(np.float32)             # [1, 1024]
    z_flat = z.reshape(B, D, HW)

    in_maps = []
    for i in range(NCORES):
        zs = np.ascontiguousarray(z_flat[i * BPC:(i + 1) * BPC])   # [4, 256, 1024]
        # sz for rows n=(b,hw) in NHWC order: z^T rows; layout [128, NT] (p, t)
        zrows = np.ascontiguousarray(zs.transpose(0, 2, 1).reshape(NPC, D))
        sz = _seqsum_sq_f32(zrows)                                 # [4096]
        szpt = np.ascontiguousarray(sz.reshape(NT, 128).T)         # [128, NT]
        in_maps.append({
            "zshard": zs,
            "zrows": zrows,
            "e2T": e2T,
            "etab": e,
            "szpt": szpt,
            "serow": se,
        })

    res = run_bass_kernel_spmd(nc, in_maps, list(range(NCORES))).results

    zq = np.concatenate([r["zq"].reshape(BPC, D, H, W) for r in res], axis=0)
    idx = np.concatenate([r["idx"].reshape(NPC) for r in res], axis=0).astype(np.int32)
    loss = np.concatenate([r["loss"].reshape(BPC, H, W, D) for r in res], axis=0)
    return zq, idx, loss
